# revision 1
# baseline (speedup 1.0000x reference)
"""Trainium2 Bass kernel for DigitConvolutionalModel (conv3x3 -> 3-layer MLP).

Strategy:
  - Pure data parallel over 8 NeuronCores: batch 65536 -> 8192 per core.
  - Host folds the 3x3 valid conv (28x28 -> 26x26) into W1:
        h1 = relu(conv(x) @ W1 + b1) = relu(x @ (C @ W1) + b1)
    where C (784, 676) is the sparse conv unfold matrix. W1f = C @ W1 is
    computed on host in float64 and cast to fp32.
  - Host pre-transposes each x shard to (784, 8192) so the contraction dim
    lies on SBUF partitions; all three layers run in transposed layout
    (h^T = W^T @ x^T), so activations stay [feat_part, batch_free] and no
    on-chip transposes are needed. The (10, 8192) output is transposed back
    on host.
  - Matmuls run as float32r (1 PE cycle/row at N=512 vs 4 for fp32);
    bias+ReLU fused into ScalarEngine activation reading PSUM.
  - Layers are software-pipelined across 512-column batch chunks
    (L1(c) | L2(c-1) | L3(c-2)) so the PE never waits on ScalarE.
"""

import os
import sys

sys.path.insert(0, "/opt/trn_rl_repo")

import numpy as np

import concourse.bass as bass
import concourse.tile as tile
from concourse import mybir
import bass_rust
from concourse.bass_utils import run_bass_kernel_spmd

NCORES = 8
B = 65536
BC = B // NCORES          # 8192 rows per core
CHUNK = 512               # moving-dim tile (one PSUM bank of fp32)
NCHUNK = BC // CHUNK      # 16

K1, NK1, K1T = 784, 7, 128     # L1 contraction tiling (K padded 784 -> 896)
K1P = NK1 * K1T                # 896
M1, NM1, M1T = 500, 4, 125     # L1 output-feature tiling
K2, NK2, K2T = 500, 4, 125
M2, NM2, M2T = 200, 2, 100
K3, NK3, K3T = 200, 2, 100
M3 = 10

F32 = mybir.dt.float32
USE_F32R = os.environ.get("KERNEL_FP32_FULL", "0") != "1"
MM_DT = mybir.dt.float32r if USE_F32R else mybir.dt.float32


def _split_excess_waits(nc, max_waits=1):
    """This walrus build caps sync-wait commands per instruction (Drain at 1).
    Hoist extra waits onto wait-only nops inserted just before, same engine."""
    ctr = 0
    for f in nc.m.functions:
        for bb in f.blocks:
            insts = bb.instructions
            i = 0
            while i < len(insts):
                inst = insts[i]
                si = inst.sync_info
                waits = list(si.on_wait) if (si and si.on_wait) else []
                if len(waits) > max_waits:
                    keep = waits[-max_waits:]
                    extra = waits[:-max_waits]
                    inst.sync_info = bass_rust.SyncInfo(
                        on_wait=keep, on_update=list(si.on_update or []))
                    nops = []
                    for j in range(0, len(extra), max_waits):
                        nop = mybir.InstNoOp(
                            name=f"WSPLIT-{ctr}", ins=[], outs=[])
                        ctr += 1
                        nop.engine = inst.engine
                        nop.sync_info = bass_rust.SyncInfo(
                            on_wait=extra[j:j + max_waits], on_update=[])
                        nops.append(nop)
                    insts[i:i] = nops
                    i += len(nops)
                i += 1
    return ctr


def build_bass():
    nc = bass.Bass(target_bir_lowering=False)
    Relu = mybir.ActivationFunctionType.Relu
    Ident = mybir.ActivationFunctionType.Identity

    xh = nc.declare_dram_parameter("xh", [NK1, K1T, BC], MM_DT, isOutput=False)
    w1 = nc.declare_dram_parameter("w1", [K1P, M1], MM_DT, isOutput=False)
    b1 = nc.declare_dram_parameter("b1", [M1], F32, isOutput=False)
    w2 = nc.declare_dram_parameter("w2", [K2, M2], MM_DT, isOutput=False)
    b2 = nc.declare_dram_parameter("b2", [M2], F32, isOutput=False)
    w3 = nc.declare_dram_parameter("w3", [K3, M3], MM_DT, isOutput=False)
    b3 = nc.declare_dram_parameter("b3", [M3], F32, isOutput=False)
    out = nc.declare_dram_parameter("out", [M3, BC], F32, isOutput=True)

    with tile.TileContext(nc) as tc:
        with (
            tc.tile_pool(name="singles", bufs=1) as singles,
            tc.tile_pool(name="xp", bufs=4) as xp,
            tc.tile_pool(name="h1p", bufs=2) as h1p,
            tc.tile_pool(name="h2p", bufs=2) as h2p,
            tc.tile_pool(name="op", bufs=2) as op,
            tc.tile_pool(name="ps1p", bufs=4, space="PSUM") as ps1p,
            tc.tile_pool(name="ps2p", bufs=2, space="PSUM") as ps2p,
            tc.tile_pool(name="ps3p", bufs=2, space="PSUM") as ps3p,
        ):
            # weights + biases via SWDGE (gpsimd) so they don't serialize
            # behind / ahead of the x prefetch stream on Sync
            w1_ks = []
            w1_k = w1.rearrange("(k p) m -> k p m", k=NK1)
            for k in range(NK1):
                w1t = singles.tile([K1T, M1], MM_DT, tag=f"w1_{k}")
                nc.gpsimd.dma_start(out=w1t, in_=w1_k[k])
                w1_ks.append(w1t)
            w2_s = singles.tile([K2T, NK2, M2], MM_DT)
            nc.gpsimd.dma_start(out=w2_s, in_=w2.rearrange("(k p) m -> p k m", k=NK2))
            w3_s = singles.tile([K3T, NK3, M3], MM_DT)
            nc.gpsimd.dma_start(out=w3_s, in_=w3.rearrange("(k p) m -> p k m", k=NK3))
            b1_s = singles.tile([M1T, NM1], F32)
            nc.gpsimd.dma_start(out=b1_s, in_=b1.rearrange("(m p) -> p m", m=NM1))
            b2_s = singles.tile([M2T, NM2], F32)
            nc.gpsimd.dma_start(out=b2_s, in_=b2.rearrange("(m p) -> p m", m=NM2))
            b3_s = singles.tile([M3, 1], F32)
            nc.gpsimd.dma_start(out=b3_s, in_=b3.rearrange("(m p) -> p m", m=1))

            # x prefetch: per-k [128, G*512] HWDGE loads with large
            # contiguous per-partition runs (measured ~350 GB/s vs ~160
            # for strided 2KB-segment shapes). Small groups first so the
            # PE starts early.
            groups = [(0, 1), (1, 1), (2, 1), (3, 1)] + \
                [(c, 2) for c in range(4, NCHUNK, 2)]
            chunk_src = {}  # chunk -> (ktiles list, column offset)

            h1_tiles = [None] * NCHUNK
            h2_tiles = [None] * NCHUNK
            for c in range(NCHUNK + 2):
                # stage 1: load x chunk c, run L1 matmuls + relu
                if c < NCHUNK:
                    for g0, glen in groups:
                        if g0 == c:
                            xks = []
                            for k in range(NK1):
                                t = xp.tile([K1T, glen * CHUNK], MM_DT,
                                            tag=f"x_{k}")
                                # startup: split issue across both HWDGE
                                # sequencers (Sync + Scalar) — descriptor
                                # generation is ~0.7us per dma_start and
                                # would otherwise pace the early chunks
                                eng = nc.scalar if (g0 < 4 and k % 2) \
                                    else nc.sync
                                eng.dma_start(
                                    out=t,
                                    in_=xh[k, :,
                                           g0 * CHUNK:(g0 + glen) * CHUNK])
                                xks.append(t)
                            for cc in range(g0, g0 + glen):
                                chunk_src[cc] = (xks, (cc - g0) * CHUNK)
                    xks, xoff = chunk_src[c]
                    rhs_of_k = lambda k, xks=xks, xoff=xoff: \
                        xks[k][:, xoff:xoff + CHUNK]
                    h1s = []
                    for m in range(NM1):
                        ps1 = ps1p.tile([M1T, CHUNK], F32)
                        for k in range(NK1):
                            nc.tensor.matmul(
                                ps1,
                                lhsT=w1_ks[k][:, m * M1T:(m + 1) * M1T],
                                rhs=rhs_of_k(k),
                                start=(k == 0), stop=(k == NK1 - 1))
                        h1 = h1p.tile([M1T, CHUNK], MM_DT, tag=f"h1_{m}")
                        nc.scalar.activation(
                            out=h1, in_=ps1, func=Relu,
                            bias=b1_s[:, m:m + 1], scale=1.0)
                        h1s.append(h1)
                    h1_tiles[c] = h1s
                # stage 2: L2 for chunk c-1
                if 1 <= c <= NCHUNK:
                    h1s = h1_tiles[c - 1]
                    h2s = []
                    for m in range(NM2):
                        ps2 = ps2p.tile([M2T, CHUNK], F32)
                        for k in range(NK2):
                            nc.tensor.matmul(
                                ps2,
                                lhsT=w2_s[:, k, m * M2T:(m + 1) * M2T],
                                rhs=h1s[k],
                                start=(k == 0), stop=(k == NK2 - 1))
                        h2 = h2p.tile([M2T, CHUNK], MM_DT, tag=f"h2_{m}")
                        nc.scalar.activation(
                            out=h2, in_=ps2, func=Relu,
                            bias=b2_s[:, m:m + 1], scale=1.0)
                        h2s.append(h2)
                    h2_tiles[c - 1] = h2s
                # stage 3: L3 for chunk c-2 + store
                if c >= 2:
                    cc = c - 2
                    h2s = h2_tiles[cc]
                    ps3 = ps3p.tile([M3, CHUNK], F32)
                    for k in range(NK3):
                        nc.tensor.matmul(
                            ps3, lhsT=w3_s[:, k, :], rhs=h2s[k],
                            start=(k == 0), stop=(k == NK3 - 1))
                    o_t = op.tile([M3, CHUNK], F32)
                    nc.scalar.activation(
                        out=o_t, in_=ps3, func=Ident,
                        bias=b3_s[:, 0:1], scale=1.0)
                    nc.gpsimd.dma_start(
                        out=out[:, cc * CHUNK:(cc + 1) * CHUNK], in_=o_t)

    _split_excess_waits(nc)
    return nc


_NC_CACHE = None


def _get_nc():
    global _NC_CACHE
    if _NC_CACHE is None:
        _NC_CACHE = build_bass()
    return _NC_CACHE


def _conv_unfold(conv_w):
    """C (784, 676): x_flat @ C == flatten(valid 3x3 xcorr of x as 28x28)."""
    C = np.zeros((784, 676), dtype=np.float64)
    w = np.asarray(conv_w, dtype=np.float64)
    for i in range(26):
        for j in range(26):
            q = 26 * i + j
            for di in range(3):
                for dj in range(3):
                    C[28 * (i + di) + (j + dj), q] += w[di, dj]
    return C


def kernel(x, conv_w, W1, b1, W2, b2, W3, b3, _trace=False, _tmpdir=None):
    x = np.asarray(x, dtype=np.float32)
    conv_w = np.asarray(conv_w, dtype=np.float32)
    W1 = np.asarray(W1, dtype=np.float32)
    b1 = np.asarray(b1, dtype=np.float32)
    W2 = np.asarray(W2, dtype=np.float32)
    b2 = np.asarray(b2, dtype=np.float32)
    W3 = np.asarray(W3, dtype=np.float32)
    b3 = np.asarray(b3, dtype=np.float32)

    C = _conv_unfold(conv_w)
    W1f = np.zeros((K1P, M1), dtype=np.float32)  # K padded 784 -> 896
    W1f[:K1] = (C @ W1.astype(np.float64)).astype(np.float32)

    nc = _get_nc()
    in_maps = []
    for c in range(NCORES):
        xh = np.zeros((NK1 * K1T, BC), dtype=np.float32)
        xh[:K1] = x[c * BC:(c + 1) * BC, :].T  # (784, 8192), zero-padded
        in_maps.append({
            "xh": xh.reshape(NK1, K1T, BC), "w1": W1f, "b1": b1,
            "w2": W2, "b2": b2, "w3": W3, "b3": b3,
        })

    res = run_bass_kernel_spmd(
        nc, in_maps, list(range(NCORES)), trace=_trace, tmpdir=_tmpdir)
    out = np.empty((B, M3), dtype=np.float32)
    for c in range(NCORES):
        out[c * BC:(c + 1) * BC, :] = res.results[c]["out"].T
    if _trace:
        return out, res
    return out

